# revision 1
# baseline (speedup 1.0000x reference)
"""Bass/Tile Trainium2 kernel for nn_Attention_9929964388721.

Module: 4-head spatial attention over [b=4, c=256, 64, 64] images.
  qkv = w_qkv @ x  (1x1 conv), split q/k/v with heads=4, dim_head=32,
  q,k l2-normalized over dim_head, sim = 10 * q^T k  (n=4096 tokens),
  attn = softmax(sim), out = attn @ v, y = w_out @ out + b_out.

Sharding (8 cores): core c handles batch b = c//2 and heads {2*(c%2), 2*(c%2)+1}.
Each core computes its partial y contribution [256, 4096]; the host sums the
two partials per batch and adds the bias.

Per-core pipeline (unit = one (batch, head) pair; 2 units/core), all matmuls
in fp16 (fp32 PSUM accumulate):
  P1: vT projection (x stationary, both units merged, ones column appended so
      the PV matmul also emits softmax row-sums); then per unit: q projection
      (replicated 4x across partition row-groups for K=32 row-packing), k
      projection into grouped layout kg[32r+d, c*128+jj] = k[d, (4c+r)*128+jj]
      via col-packed matmuls with strided moving x, sum-of-squares via
      block-ones matmuls, rsqrt as exp(-0.5*ln(x)) on ACT, scale broadcast +
      apply on DVE (softmax scale 10 folded into rq).
  P2 (ACT-bound steady state, ~72% of runtime): per (unit, i-chunk of 512):
      32 j-tile sim matmuls 4-way row-packed (tile_position) into alternating
      4-bank/3-bank PSUM tiles, one exp per tile on ACT (PSUM -> SBUF fp16),
      32 accumulating PV matmuls (M=33) into one PSUM bank. Emission is
      software-pipelined one group ahead so the in-order PE queue never
      head-blocks on ACT; the even group count keeps the A/B PSUM slots
      strictly alternating across chunk boundaries.
  P3: per-unit rowsum reciprocal + broadcast, oT scaling and output
      projection pipelined per chunk.
"""

import sys

sys.path.insert(0, "/opt/trn_rl_repo")

from contextlib import ExitStack

import numpy as np

import concourse.mybir as mybir
import concourse.tile as tile
from concourse import bacc
from concourse.bass_utils import run_bass_kernel_spmd

HEADS = 4
DIM_HEAD = 32
B, C, H, W = 4, 256, 64, 64
N = H * W                  # 4096 tokens
HIDDEN = HEADS * DIM_HEAD  # 128
NCORES = 8
UNITS = 2                  # (batch, head) pairs per core
CHUNK = 512                # i-chunk width
NCHUNK = N // CHUNK        # 8
JT = N // 128              # 32 j-tiles of 128
F32 = mybir.dt.float32
F16 = mybir.dt.float16
LN10 = float(np.log(10.0))

# Sim/exp PSUM ping-pong group sizes. Even count with strict A/B tag
# alternation (A: 4-bank slot, B: 3-bank slot) so consecutive groups always
# use different slots, including across chunk boundaries.
# Banks: A(4) + B(3) + PV accumulator(1) = 8.
GROUPS = [4, 3, 4, 3, 4, 3, 4, 3, 2, 2]
assert sum(GROUPS) == JT and len(GROUPS) % 2 == 0


def _build():
    nc = bacc.Bacc("TRN2", target_bir_lowering=False, debug=False,
                   num_devices=NCORES)

    # ---- DRAM I/O ----
    x_in = nc.dram_tensor("x_in", [C, N], F16, kind="ExternalInput").ap()
    wqT = nc.dram_tensor("wqT", [UNITS, C, 128], F16, kind="ExternalInput").ap()
    wkT = nc.dram_tensor("wkT", [UNITS, C, DIM_HEAD], F16,
                         kind="ExternalInput").ap()
    wvT = nc.dram_tensor("wvT", [C, 2 * DIM_HEAD], F16,
                         kind="ExternalInput").ap()
    woT = nc.dram_tensor("woT", [2 * DIM_HEAD, 2, 128], F16,
                         kind="ExternalInput").ap()
    ones4 = nc.dram_tensor("ones4", [128, 4], F16, kind="ExternalInput").ap()
    y_out = nc.dram_tensor("y_out", [C, N], F32, kind="ExternalOutput").ap()

    with ExitStack() as top:
        tc = top.enter_context(tile.TileContext(nc))
        persist = top.enter_context(tc.tile_pool(name="persist", bufs=1))
        p12 = top.enter_context(ExitStack())
        chains = p12.enter_context(tc.tile_pool(name="chains", bufs=1))
        dram = top.enter_context(tc.tile_pool(name="dram", bufs=1, space="DRAM"))

        qs = [persist.tile([128, N], F16, name=f"qs{u}", tag=f"qs{u}")
              for u in range(UNITS)]
        kg = [persist.tile([128, N // 4], F16, name=f"kg{u}", tag=f"kg{u}")
              for u in range(UNITS)]
        kg_raw = [persist.tile([128, N // 4], F32, name=f"kg_raw{u}",
                               tag=f"kg_raw{u}") for u in range(UNITS)]
        vt = [persist.tile([128, JT, DIM_HEAD + 1], F16, name=f"vt{u}",
                           tag=f"vt{u}") for u in range(UNITS)]
        # av_sb[u]: rows 0..31 = attn-out^T (d, i), row 32 = rowsums
        av_sb = [persist.tile([DIM_HEAD + 1, N], F32, name=f"av_sb{u}",
                              tag=f"av_sb{u}") for u in range(UNITS)]
        w_o = persist.tile([DIM_HEAD, UNITS, 2, 128], F16, name="w_o",
                           tag="w_o")
        woT_v = woT.rearrange("(u kk) m mm -> kk u m mm", u=UNITS)
        for u in range(UNITS):
            nc.sync.dma_start(out=w_o[:, u, :, :], in_=woT_v[:, u, :, :])

        # =========================== P1: projections =======================
        # PSUM pool creation order fixes bank addresses:
        #   pq 0-1, pssq 2-3, pk 4-5, pv 6-7.
        # P2 then gets stA -> banks 0-3 (freed by the last q/ssq use), stB ->
        # 4-6, av -> 7.
        with ExitStack() as p1:
            wpool = p1.enter_context(tc.tile_pool(name="wpool", bufs=1))
            sc = p1.enter_context(tc.tile_pool(name="p1scratch", bufs=2))
            pq = p1.enter_context(tc.tile_pool(name="pq", bufs=2, space="PSUM"))
            pssq = p1.enter_context(tc.tile_pool(name="pssq", bufs=2,
                                                 space="PSUM"))
            pk = p1.enter_context(tc.tile_pool(name="pk", bufs=2, space="PSUM"))
            pv = p1.enter_context(tc.tile_pool(name="pv", bufs=2, space="PSUM"))

            w_q = wpool.tile([128, 2, UNITS, 128], F16, name="w_q", tag="w_q")
            w_k = wpool.tile([128, 2, UNITS, DIM_HEAD], F16, name="w_k",
                             tag="w_k")
            w_v = wpool.tile([128, 2, 2 * DIM_HEAD], F16, name="w_v", tag="w_v")
            for dst, srct in ((w_q, wqT), (w_k, wkT)):
                srcv = srct.rearrange("u (kt p) m -> p kt u m", p=128)
                for kt in range(2):
                    for u in range(UNITS):
                        nc.sync.dma_start(out=dst[:, kt, u, :],
                                          in_=srcv[:, kt, u, :])
            nc.sync.dma_start(out=w_v[:, :, :],
                              in_=wvT.rearrange("(kt p) m -> p kt m", p=128))
            o4 = wpool.tile([128, 4], F16, name="o4", tag="o4")
            nc.sync.dma_start(out=o4[:, :], in_=ones4)

            x_sb = wpool.tile([128, 2, N], F16, name="x_sb", tag="x_sb")
            x_view = x_in.rearrange("(kt p) n -> p kt n", p=128)
            for ch in range(NCHUNK):
                for kt in range(2):
                    nc.sync.dma_start(
                        out=x_sb[:, kt, ch * CHUNK:(ch + 1) * CHUNK],
                        in_=x_view[:, kt, ch * CHUNK:(ch + 1) * CHUNK])

            # --- vT projection (x stationary, units merged, N=64) ---
            for u in range(UNITS):
                nc.vector.memset(vt[u][:, :, :], 1.0)
            for jt in range(JT):
                ps = pv.tile([128, 2 * DIM_HEAD], F32, name="psv", tag="psv")
                for kt in range(2):
                    nc.tensor.matmul(
                        ps[:, :],
                        x_sb[:, kt, jt * 128:(jt + 1) * 128],
                        w_v[:, kt, :],
                        start=(kt == 0), stop=(kt == 1))
                for u in range(UNITS):
                    nc.vector.tensor_copy(
                        vt[u][:, jt, 0:DIM_HEAD],
                        ps[:, u * DIM_HEAD:(u + 1) * DIM_HEAD])

            q_rep = [chains.tile([128, N], F32, name=f"q_rep{u}",
                                 tag=f"q_rep{u}") for u in range(UNITS)]
            # rr_d[u, 0] = rq values (r, ch, jj); rr_d[u, 1] = rk (r, c, jj)
            # rr_d[u, a] flat in i order (= c*512 + r*128 + jj)
            rr_d = dram.tile([UNITS, 2, N], F32, name="rr_d", tag="rr_d")
            ln10_t = chains.tile([4, 1], F32, name="ln10_t", tag="ln10_t")
            nc.vector.memset(ln10_t[:, :], LN10)

            sstq = [chains.tile([4, NCHUNK, 128], F32, name=f"sstq{u}",
                             tag=f"sstq{u}") for u in range(UNITS)]
            sstk = [chains.tile([4, NCHUNK, 128], F32, name=f"sstk{u}",
                             tag=f"sstk{u}") for u in range(UNITS)]

            # ---- per-unit: phase A (PSUM work) then phase B (norm chain) ----
            for u in range(UNITS):
                # q replicated projection (for the sim matmuls)
                for ch in range(NCHUNK):
                    ps = pq.tile([128, CHUNK], F32, name="psq", tag="psq")
                    for kt in range(2):
                        nc.tensor.matmul(
                            ps[:, :],
                            w_q[:, kt, u, :],
                            x_sb[:, kt, ch * CHUNK:(ch + 1) * CHUNK],
                            start=(kt == 0), stop=(kt == 1))
                    nc.scalar.copy(
                        q_rep[u][:, ch * CHUNK:(ch + 1) * CHUNK], ps[:, :])

                # grouped q projection, only for its sum-of-squares
                for h in range(2):
                    ps = pk.tile([128, CHUNK], F32, name="psk", tag="psk")
                    for r in range(4):
                        for kt in range(2):
                            xv = x_sb[:, kt, :].rearrange(
                                "p (blk cc jj) -> p blk cc jj", cc=4, jj=128)
                            nc.tensor.matmul(
                                ps[32 * r:32 * r + 32, :],
                                w_q[:, kt, u, 0:DIM_HEAD],
                                xv[:, 4 * h:4 * h + 4, r, :],
                                start=(kt == 0), stop=(kt == 1),
                                tile_position=(0, 32 * r))
                    sq = sc.tile([128, CHUNK], F16, name="sq", tag="sq")
                    if u == 0:
                        qg_sb = sc.tile([128, CHUNK], F32, name="qg_sb",
                                        tag="qg_sb")
                        nc.vector.tensor_copy(qg_sb[:, :], ps[:, :])
                        nc.vector.tensor_mul(sq[:, :], qg_sb[:, :],
                                             qg_sb[:, :])
                    else:
                        nc.scalar.activation(
                            sq[:, :], ps[:, :],
                            mybir.ActivationFunctionType.Square)
                    ps2 = pssq.tile([4, CHUNK], F32, name="psssq", tag="psssq")
                    nc.tensor.matmul(ps2[:, :], o4[:, :], sq[:, :],
                                     start=True, stop=True)
                    nc.vector.tensor_copy(
                        sstq[u][:, 4 * h:4 * h + 4, :],
                        ps2[:, :].rearrange("r (cc jj) -> r cc jj", jj=128))

                # k grouped projection + ssq
                for h in range(2):
                    ps = pk.tile([128, CHUNK], F32, name="psk", tag="psk")
                    for r in range(4):
                        for kt in range(2):
                            xv = x_sb[:, kt, :].rearrange(
                                "p (blk cc jj) -> p blk cc jj", cc=4, jj=128)
                            nc.tensor.matmul(
                                ps[32 * r:32 * r + 32, :],
                                w_k[:, kt, u, :],
                                xv[:, 4 * h:4 * h + 4, r, :],
                                start=(kt == 0), stop=(kt == 1),
                                tile_position=(0, 32 * r))
                    nc.vector.tensor_copy(
                        kg_raw[u][:, h * CHUNK:(h + 1) * CHUNK], ps[:, :])
                    sq = sc.tile([128, CHUNK], F16, name="sq", tag="sq")
                    if u == 0:
                        kr = kg_raw[u][:, h * CHUNK:(h + 1) * CHUNK]
                        nc.vector.tensor_mul(sq[:, :], kr, kr)
                    else:
                        nc.scalar.activation(
                            sq[:, :], ps[:, :],
                            mybir.ActivationFunctionType.Square)
                    ps2 = pssq.tile([4, CHUNK], F32, name="psssq", tag="psssq")
                    nc.tensor.matmul(ps2[:, :], o4[:, :], sq[:, :],
                                     start=True, stop=True)
                    nc.vector.tensor_copy(
                        sstk[u][:, 4 * h:4 * h + 4, :],
                        ps2[:, :].rearrange("r (cc jj) -> r cc jj", jj=128))

                nc.scalar.activation(sstq[u][:, :, :], sstq[u][:, :, :],
                                     mybir.ActivationFunctionType.Ln)
                nc.scalar.activation(sstk[u][:, :, :], sstk[u][:, :, :],
                                     mybir.ActivationFunctionType.Ln)
                # rq = 10/|q| = exp(-0.5*ln(ssq) + ln 10);  rk = 1/|k|
                nc.scalar.activation(sstq[u][:, :, :], sstq[u][:, :, :],
                                     mybir.ActivationFunctionType.Exp,
                                     bias=ln10_t[:, :], scale=-0.5)
                nc.scalar.activation(sstk[u][:, :, :], sstk[u][:, :, :],
                                     mybir.ActivationFunctionType.Exp,
                                     bias=0.0, scale=-0.5)
                for a, sst in ((0, sstq[u]), (1, sstk[u])):
                    nc.gpsimd.dma_start(
                        out=rr_d[u, a, :].rearrange("(c r jj) -> r c jj",
                                                    r=4, jj=128),
                        in_=sst[:, :, :])

                rqb = chains.tile([128, N], F32, name="rqb", tag="rqb")
                if u == 0:
                    # broadcast on GPSIMD, quartered for pipelining
                    rq_row = chains.tile([1, N], F32, name="rq_row",
                                         tag="rq_row")
                    nc.sync.dma_start(out=rq_row[:, :],
                                      in_=rr_d[u, 0, :].unsqueeze(0))
                    for hh in range(4):
                        hs = slice(hh * (N // 4), (hh + 1) * (N // 4))
                        nc.gpsimd.partition_broadcast(rqb[:, hs],
                                                      rq_row[:, hs])
                        nc.vector.tensor_mul(qs[u][:, hs], q_rep[u][:, hs],
                                             rqb[:, hs])
                else:
                    # broadcast-DMAs (overlap the running mainloop)
                    for ch in range(NCHUNK):
                        eng = nc.sync if ch % 2 == 0 else nc.gpsimd
                        eng.dma_start(
                            out=rqb[:, ch * CHUNK:(ch + 1) * CHUNK],
                            in_=rr_d[u, 0, ch * CHUNK:(ch + 1) * CHUNK]
                            .partition_broadcast(128))
                    for hh in range(4):
                        hs = slice(hh * (N // 4), (hh + 1) * (N // 4))
                        nc.vector.tensor_mul(qs[u][:, hs], q_rep[u][:, hs],
                                             rqb[:, hs])
                rkb = chains.tile([128, N // 4], F32, name="rkb", tag="rkb")
                rkv = rr_d[u, 1, :].rearrange("(c r jj) -> r c jj", r=4,
                                              jj=128)
                for r in range(4):
                    eng = nc.sync if r % 2 == 0 else nc.gpsimd
                    eng.dma_start(
                        out=rkb[32 * r:32 * r + 32, :].rearrange(
                            "p (c jj) -> p c jj", jj=128),
                        in_=rkv[r, :, :].partition_broadcast(32))
                for hh in range(2):
                    hs = slice(hh * (N // 8), (hh + 1) * (N // 8))
                    nc.vector.tensor_mul(kg[u][:, hs], kg_raw[u][:, hs],
                                         rkb[:, hs])

        # =========================== P2: attention =========================
        with ExitStack() as p2:
            pst = p2.enter_context(tc.tile_pool(name="pst", bufs=1,
                                                space="PSUM"))
            pav = p2.enter_context(tc.tile_pool(name="pav", bufs=1,
                                                space="PSUM"))
            ptp = p2.enter_context(tc.tile_pool(name="ptp", bufs=2))

            # Software pipeline with one-group lookahead: PE program order is
            # sim(0), sim(1), av(0), sim(2), av(1), ... so the PE never
            # head-blocks on ACT(g) with ready sim work behind it.
            pending = None

            def emit_act_av(p):
                u, st, pt, jt0, g, av, last, i0 = p
                nc.scalar.activation(pt[:, :, :], st[:, :, :],
                                     mybir.ActivationFunctionType.Exp)
                for s in range(g):
                    j = jt0 + s
                    nc.tensor.matmul(
                        av[:, :],
                        vt[u][:, j, :],
                        pt[:, s, :],
                        start=(j == 0), stop=(j == JT - 1))
                if last:
                    nc.vector.tensor_copy(av_sb[u][:, i0:i0 + CHUNK], av[:, :])

            for u in range(UNITS):
                for ch in range(NCHUNK):
                    i0 = ch * CHUNK
                    av = pav.tile([DIM_HEAD + 1, CHUNK], F32, name="av",
                                  tag="av")
                    jt = 0
                    for gidx, g in enumerate(GROUPS):
                        tag = "A" if gidx % 2 == 0 else "B"
                        st = pst.tile([128, g, CHUNK], F32, name=f"st{tag}",
                                      tag=f"st{tag}")
                        pt = ptp.tile([128, g, CHUNK], F16, name=f"pt{tag}",
                                      tag=f"pt{tag}")
                        for s in range(g):
                            j = jt + s
                            r = j % 4
                            t = j // 4
                            nc.tensor.matmul(
                                st[:, s, :],
                                kg[u][32 * r:32 * r + 32,
                                      t * 128:(t + 1) * 128],
                                qs[u][32 * r:32 * r + 32, i0:i0 + CHUNK],
                                start=True, stop=True,
                                tile_position=(32 * r, 0))
                        if pending is not None:
                            emit_act_av(pending)
                        pending = (u, st, pt, jt, g, av,
                                   gidx == len(GROUPS) - 1, i0)
                        jt += g
            emit_act_av(pending)

        p12.close()

        # =========================== P3: epilogue ==========================
        with ExitStack() as p3:
            sc3 = p3.enter_context(tc.tile_pool(name="p3scratch", bufs=2))
            py = p3.enter_context(tc.tile_pool(name="py", bufs=4, space="PSUM"))

            rcp_d = dram.tile([UNITS, 2, N // 2], F32, name="rcp_d",
                              tag="rcp_d")
            oT = [sc3.tile([DIM_HEAD, N], F16, name=f"oT{u}", tag=f"oT{u}",
                           bufs=1) for u in range(UNITS)]
            rsb = [sc3.tile([DIM_HEAD, N], F32, name=f"rsb{u}", tag=f"rsb{u}",
                            bufs=1) for u in range(UNITS)]
            # per-(unit, half) chains + early oT muls; everything except
            # unit1's second half overlaps the mainloop
            for u in range(UNITS):
                for hf in range(2):
                    fs = slice(hf * (N // 2), (hf + 1) * (N // 2))
                    rsq = sc3.tile([128, N // 256], F32, name="rsq", tag="rsq")
                    nc.gpsimd.dma_start(
                        out=rsq[:, :],
                        in_=av_sb[u][DIM_HEAD:DIM_HEAD + 1, fs])
                    rcp = sc3.tile([128, N // 256], F32, name="rcp", tag="rcp")
                    nc.vector.reciprocal(rcp[:, :], rsq[:, :])
                    nc.gpsimd.dma_start(
                        out=rcp_d[u, hf, :].rearrange("(p f) -> p f", p=128),
                        in_=rcp[:, :])
                    for qq in range(2):
                        eng = nc.sync if qq % 2 == 0 else nc.gpsimd
                        qs_ = slice(hf * (N // 2) + qq * (N // 4),
                                    hf * (N // 2) + (qq + 1) * (N // 4))
                        nc.sync.dma_start(
                            out=rsb[u][:, qs_],
                            in_=rcp_d[u, hf, qq * (N // 4):(qq + 1) * (N // 4)]
                            .partition_broadcast(DIM_HEAD))
                    for ch in range(4 * hf, 4 * hf + 4):
                        cs = slice(ch * CHUNK, (ch + 1) * CHUNK)
                        nc.vector.tensor_mul(oT[u][:, cs],
                                             av_sb[u][0:DIM_HEAD, cs],
                                             rsb[u][:, cs])

            engs = [nc.sync, nc.gpsimd, nc.scalar]
            for ch in range(NCHUNK):
                cs = slice(ch * CHUNK, (ch + 1) * CHUNK)
                for m in range(2):
                    ps = py.tile([128, CHUNK], F32, name="psy", tag="psy")
                    for u in range(UNITS):
                        nc.tensor.matmul(
                            ps[:, :],
                            w_o[:, u, m, :],
                            oT[u][:, cs],
                            start=(u == 0), stop=(u == 1))
                    ysb = sc3.tile([128, CHUNK], F32, name="ysb", tag="ysb",
                                   bufs=4)
                    ceng = nc.vector if (ch + m) % 2 == 0 else nc.scalar
                    ceng.tensor_copy(ysb[:, :], ps[:, :]) \
                        if ceng is nc.vector else ceng.copy(ysb[:, :], ps[:, :])
                    engs[(2 * ch + m) % 3].dma_start(
                        out=y_out[m * 128:(m + 1) * 128, cs], in_=ysb[:, :])

    nc.compile()
    return nc


_NC_CACHE = None


def _get_nc():
    global _NC_CACHE
    if _NC_CACHE is None:
        _NC_CACHE = _build()
    return _NC_CACHE


def _make_in_maps(x, w_qkv, w_out):
    """Build the 8 per-core input dicts from full inputs."""
    x = np.ascontiguousarray(x, dtype=np.float32)
    w_qkv = np.ascontiguousarray(w_qkv, dtype=np.float32)
    w_out = np.ascontiguousarray(w_out, dtype=np.float32)
    b, c, h, w = x.shape
    xf = x.reshape(b, c, h * w)

    ones4 = np.zeros((128, 4), np.float16)
    for r in range(4):
        ones4[32 * r:32 * r + 32, r] = 1.0

    in_maps = []
    for core in range(NCORES):
        bb = core // 2
        p = core % 2
        heads = [2 * p, 2 * p + 1]
        wq = np.stack([w_qkv[hh * DIM_HEAD:(hh + 1) * DIM_HEAD, :]
                       for hh in heads])
        wk = np.stack([w_qkv[HIDDEN + hh * DIM_HEAD:
                             HIDDEN + (hh + 1) * DIM_HEAD, :] for hh in heads])
        wv = np.stack([w_qkv[2 * HIDDEN + hh * DIM_HEAD:
                             2 * HIDDEN + (hh + 1) * DIM_HEAD, :]
                       for hh in heads])
        wqT = np.ascontiguousarray(
            np.concatenate([np.transpose(wq, (0, 2, 1))] * 4, axis=2))
        wkT = np.ascontiguousarray(np.transpose(wk, (0, 2, 1)))
        wvT = np.ascontiguousarray(
            np.concatenate([wv[0].T, wv[1].T], axis=1))  # [256, 64]
        wo_cols = np.concatenate(
            [w_out[:, heads[0] * DIM_HEAD:(heads[0] + 1) * DIM_HEAD],
             w_out[:, heads[1] * DIM_HEAD:(heads[1] + 1) * DIM_HEAD]], axis=1)
        woT = np.ascontiguousarray(wo_cols.T.reshape(64, 2, 128))
        in_maps.append({
            "x_in": np.ascontiguousarray(xf[bb]).astype(np.float16),
            "wqT": wqT.astype(np.float16),
            "wkT": wkT.astype(np.float16),
            "wvT": wvT.astype(np.float16),
            "woT": woT.astype(np.float16),
            "ones4": ones4,
        })
    return in_maps


def kernel(x, w_qkv, w_out, b_out):
    nc = _get_nc()
    in_maps = _make_in_maps(x, w_qkv, w_out)
    res = run_bass_kernel_spmd(nc, in_maps, core_ids=list(range(NCORES)))
    outs = res.results
    y = np.zeros((B, C, N), np.float32)
    for bb in range(B):
        y[bb] = outs[2 * bb]["y_out"] + outs[2 * bb + 1]["y_out"]
    y += np.asarray(b_out, np.float32)[None, :, None]
    return y.reshape(B, C, H, W).astype(np.float32)



# revision 12
# speedup vs baseline: 1.0922x; 1.0922x over previous
"""Bass/Tile Trainium2 kernel for nn_Attention_9929964388721.

Module: 4-head spatial attention over [b=4, c=256, 64, 64] images.
  qkv = w_qkv @ x  (1x1 conv), split q/k/v with heads=4, dim_head=32,
  q,k l2-normalized over dim_head, sim = 10 * q^T k  (n=4096 tokens),
  attn = softmax(sim), out = attn @ v, y = w_out @ out + b_out.

Sharding (8 cores): core c handles batch b = c//2 and heads {2*(c%2), 2*(c%2)+1}.
Each core computes its partial y contribution [256, 4096]; the host sums the
two partials per batch and adds the bias.

v2 pipeline (exp split across ACT and DVE, col-packed PV):
  - qs is pre-scaled by 10*1024*log2(e)/|q| so the sim matmul emits
    t = 1024*log2(e)*sim directly. ACT tiles: exp via activation
    (scale=ln2/1024). DVE tiles: custom fast-exp op emits fp16 bit patterns
    (round-split + quadratic mantissa fill + exact 2x negative-branch rule,
    max rel err ~2.3e-3) via int16 output convert at 1 elem/cycle.
  - PV: per j-tile an M=32 av matmul (col group j%4, tile_position) plus an
    M=1 rowsum matmul (ones/1024 stationary, v pre-scaled by 2^-10 so
    av/rowsum scales cancel). 4 col groups stream concurrently.
  - Per chunk: rowsum partials folded via a sel matmul (rows 0/32/64/96),
    RECIPROCAL_APPROX_FAST on DVE, oT = av * (1/S) into fp16.
  - Tail: y = wo4^T @ oT per chunk/half, K=128 matmuls accumulate both units
    and the 4 col-group partials in one pass.
"""

import sys

sys.path.insert(0, "/opt/trn_rl_repo")

from contextlib import ExitStack

import numpy as np

import concourse.mybir as mybir
import concourse.tile as tile
from concourse import bacc
from concourse.bass_utils import run_bass_kernel_spmd

HEADS = 4
DIM_HEAD = 32
B, C, H, W = 4, 256, 64, 64
N = H * W                  # 4096 tokens
HIDDEN = HEADS * DIM_HEAD  # 128
NCORES = 8
UNITS = 2                  # (batch, head) pairs per core
CHUNK = 512                # i-chunk width
NCHUNK = N // CHUNK        # 8
JT = N // 128              # 32 j-tiles of 128
F32 = mybir.dt.float32
F16 = mybir.dt.float16
I16 = mybir.dt.int16

A_PRE = 1024.0 * float(np.log2(np.e))     # folded into rq
LNPRE = float(np.log(10.0 * A_PRE))       # ln(10 * A_PRE) for the rq chain
LN2_1024 = float(np.log(2.0) / 1024.0)    # ACT exp scale
# floor-mode fast-exp: v = t + C1 lands in [2^23, 2^24) (exp byte 0x96);
# AND with MASK clears the low 10 mantissa bits -> 1024-aligned floor.
EXP_C1 = 12598272.0                       # 3*2^22 + 15360
EXP_MASK = float(np.uint32(0x4B7FFC00).view(np.float32))
EXP_B1 = 0.6659735417740476               # mantissa-fill poly on [0, 1024)
EXP_B2 = 0.0003221902763099779
VS = 2.0**-10                             # v / rowsum scale (cancels exactly)

# ---- custom DVE fast-exp op registration ---------------------------------


def _register_fast_exp():
    from concourse import dve_ops as dops
    from concourse.dve_spec import (
        C0, C1, C2, C3, AluOp, Bin, Spec, Src0, lower,
        _spill_c3_to_src1,
    )
    from concourse.dve_uop import DveOpSpec

    name = "FAST_EXP2_BITS_ANT"
    for op in dops.OPS:
        if op.name == name:
            return op

    # body (7 ALU ops): v = t + C1; rA = v & C2(mask); f = v - rA;
    # pm = f*(f*C3 + C0); out = pm + rA.  Written as int32; the low 16 bits
    # are the fp16 pattern because C1's 1024-aligned base is 192*65536+15360.
    v = Src0 + C1
    rA = Bin(AluOp.BITWISE_AND, v, C2)
    f = v - rA
    pm = f * (f * C3 + C0)
    body = pm + rA

    def _ref(in0, in1, s0, s1, imm2):
        t = in0.astype(np.float32)
        vv = (t + np.float32(s1)).astype(np.float32)
        mask = np.float32(imm2).view(np.uint32)
        ra = (vv.view(np.uint32) & mask).view(np.float32)
        ff = (vv - ra).astype(np.float32)
        b2 = in1.reshape(in1.shape[0], 1).astype(np.float32)
        p = ff * (ff * b2 + np.float32(s0))
        return p + ra

    spec = Spec(body=_spill_c3_to_src1(body), reference=_ref)
    row = dops._CUSTOM_DVE_ROW_BASE + len(dops.OPS)
    shas = {}
    for ver in ("v3", "v4"):
        tmp = DveOpSpec(name=name, opcode=row, uops=lower(spec, ver=ver),
                        rd1_en=True)
        shas[ver] = tmp.sha(ver)
    op = dops.DveOp(name, spec, subdim=False, uops_sha=shas)
    dops.OPS.append(op)
    dops._SUB_OPCODE_FOR_NAME[name] = row
    dops.CUSTOM_DVE_SPECS[name] = spec
    return op


FAST_EXP_OP = _register_fast_exp()


def _p2_groups(start_slot):
    """Group sizes for one chunk: alternate slot A (3 banks) / B (2 banks)
    until 32 tiles are consumed. Returns list of (size, slot)."""
    out = []
    rem, slot = JT, start_slot
    while rem:
        g = min(rem, 3 if slot == 0 else 2)
        out.append((g, slot))
        rem -= g
        slot ^= 1
    return out


def _engine_plan(groups, act_frac=0.53):
    """Greedy per-group ACT/DVE assignment targeting act_frac of tiles."""
    plan = []
    act, tot = 0, 0
    for g, _ in groups:
        if (act + g) <= act_frac * (tot + g) + 0.5:
            plan.append("ACT")
            act += g
        else:
            plan.append("DVE")
        tot += g
    return plan


def _build():
    from concourse.dve_ops import RECIPROCAL_APPROX_FAST, RECIP_APPROX_FAST_CONSTS

    nc = bacc.Bacc("TRN2", target_bir_lowering=False, debug=False,
                   num_devices=NCORES)

    # ---- DRAM I/O ----
    x_in = nc.dram_tensor("x_in", [C, N], F16, kind="ExternalInput").ap()
    wqT = nc.dram_tensor("wqT", [UNITS, C, 128], F16, kind="ExternalInput").ap()
    wkT = nc.dram_tensor("wkT", [UNITS, C, DIM_HEAD], F16,
                         kind="ExternalInput").ap()
    wvT = nc.dram_tensor("wvT", [C, 2 * DIM_HEAD], F16,
                         kind="ExternalInput").ap()
    wo4 = nc.dram_tensor("wo4", [UNITS, 128, 2, 128], F16,
                         kind="ExternalInput").ap()
    ones4 = nc.dram_tensor("ones4", [128, 4], F16, kind="ExternalInput").ap()
    y_out = nc.dram_tensor("y_out", [C, N], F32, kind="ExternalOutput").ap()

    with ExitStack() as top:
        tc = top.enter_context(tile.TileContext(nc))
        persist = top.enter_context(tc.tile_pool(name="persist", bufs=1))
        p12 = top.enter_context(ExitStack())
        chains = p12.enter_context(tc.tile_pool(name="chains", bufs=1))
        dram = top.enter_context(tc.tile_pool(name="dram", bufs=1, space="DRAM"))

        qs = [persist.tile([128, N], F16, name=f"qs{u}", tag=f"qs{u}")
              for u in range(UNITS)]
        kg = [persist.tile([128, N // 4], F16, name=f"kg{u}", tag=f"kg{u}")
              for u in range(UNITS)]
        kg_raw = [persist.tile([128, N // 4], F32, name=f"kg_raw{u}",
                               tag=f"kg_raw{u}") for u in range(UNITS)]
        vt = [persist.tile([128, JT, DIM_HEAD], F16, name=f"vt{u}",
                           tag=f"vt{u}") for u in range(UNITS)]
        oT = [persist.tile([128, N], F16, name=f"oT{u}", tag=f"oT{u}")
              for u in range(UNITS)]
        w_o = persist.tile([128, UNITS, 2, 128], F16, name="w_o", tag="w_o")
        for u in range(UNITS):
            nc.sync.dma_start(out=w_o[:, u, :, :], in_=wo4[u, :, :, :])
        # fold selector: rows 0/32/64/96 all-ones
        sel = persist.tile([128, 128], F16, name="sel", tag="sel")
        nc.vector.memset(sel[:, :], 0.0)
        for c4 in range(4):
            nc.vector.memset(sel[32 * c4:32 * c4 + 1, :], 1.0)
        # rowsum stationary: single ones column scaled by VS
        ones_rs = persist.tile([128, 1], F16, name="ones_rs", tag="ones_rs")
        nc.vector.memset(ones_rs[:, :], VS)
        # C3 spill value for the fast-exp op
        b2t = persist.tile([128, 1], F32, name="b2t", tag="b2t")
        nc.vector.memset(b2t[:, :], EXP_B2)

        # =========================== P1: projections =======================
        # PSUM pool creation order fixes bank addresses:
        #   pq 0-1, pssq 2-3, pk 4-5, pv 6-7.
        with ExitStack() as p1:
            wpool = p1.enter_context(tc.tile_pool(name="wpool", bufs=1))
            sc = p1.enter_context(tc.tile_pool(name="p1scratch", bufs=2))
            pq = p1.enter_context(tc.tile_pool(name="pq", bufs=2, space="PSUM"))
            pssq = p1.enter_context(tc.tile_pool(name="pssq", bufs=2,
                                                 space="PSUM"))
            pk = p1.enter_context(tc.tile_pool(name="pk", bufs=2, space="PSUM"))
            pv = p1.enter_context(tc.tile_pool(name="pv", bufs=2, space="PSUM"))

            w_q = wpool.tile([128, 2, UNITS, 128], F16, name="w_q", tag="w_q")
            w_k = wpool.tile([128, 2, UNITS, DIM_HEAD], F16, name="w_k",
                             tag="w_k")
            w_v = wpool.tile([128, 2, 2 * DIM_HEAD], F16, name="w_v", tag="w_v")
            for dst, srct in ((w_q, wqT), (w_k, wkT)):
                srcv = srct.rearrange("u (kt p) m -> p kt u m", p=128)
                for kt in range(2):
                    for u in range(UNITS):
                        nc.sync.dma_start(out=dst[:, kt, u, :],
                                          in_=srcv[:, kt, u, :])
            nc.sync.dma_start(out=w_v[:, :, :],
                              in_=wvT.rearrange("(kt p) m -> p kt m", p=128))
            o4 = wpool.tile([128, 4], F16, name="o4", tag="o4")
            nc.sync.dma_start(out=o4[:, :], in_=ones4)

            x_sb = wpool.tile([128, 2, N], F16, name="x_sb", tag="x_sb")
            x_view = x_in.rearrange("(kt p) n -> p kt n", p=128)
            for ch in range(NCHUNK):
                for kt in range(2):
                    nc.sync.dma_start(
                        out=x_sb[:, kt, ch * CHUNK:(ch + 1) * CHUNK],
                        in_=x_view[:, kt, ch * CHUNK:(ch + 1) * CHUNK])

            # --- vT projection (x stationary, units merged, N=64) ---
            for jt in range(JT):
                ps = pv.tile([128, 2 * DIM_HEAD], F32, name="psv", tag="psv")
                for kt in range(2):
                    nc.tensor.matmul(
                        ps[:, :],
                        x_sb[:, kt, jt * 128:(jt + 1) * 128],
                        w_v[:, kt, :],
                        start=(kt == 0), stop=(kt == 1))
                for u in range(UNITS):
                    nc.vector.tensor_copy(
                        vt[u][:, jt, :],
                        ps[:, u * DIM_HEAD:(u + 1) * DIM_HEAD])

            q_rep = [chains.tile([128, N], F32, name=f"q_rep{u}",
                                 tag=f"q_rep{u}") for u in range(UNITS)]
            # rr_d[u, 0] = rq values (r, ch, jj); rr_d[u, 1] = rk (r, c, jj)
            rr_d = dram.tile([UNITS, 2, N], F32, name="rr_d", tag="rr_d")
            lnpre_t = chains.tile([4, 1], F32, name="lnpre_t", tag="lnpre_t")
            nc.vector.memset(lnpre_t[:, :], LNPRE)

            sstq = [chains.tile([4, NCHUNK, 128], F32, name=f"sstq{u}",
                             tag=f"sstq{u}") for u in range(UNITS)]
            sstk = [chains.tile([4, NCHUNK, 128], F32, name=f"sstk{u}",
                             tag=f"sstk{u}") for u in range(UNITS)]

            # ---- per-unit: phase A (PSUM work) then phase B (norm chain) ----
            for u in range(UNITS):
                # q replicated projection (for the sim matmuls)
                for ch in range(NCHUNK):
                    ps = pq.tile([128, CHUNK], F32, name="psq", tag="psq")
                    for kt in range(2):
                        nc.tensor.matmul(
                            ps[:, :],
                            w_q[:, kt, u, :],
                            x_sb[:, kt, ch * CHUNK:(ch + 1) * CHUNK],
                            start=(kt == 0), stop=(kt == 1))
                    nc.scalar.copy(
                        q_rep[u][:, ch * CHUNK:(ch + 1) * CHUNK], ps[:, :])

                # grouped q projection, only for its sum-of-squares
                for h in range(2):
                    ps = pk.tile([128, CHUNK], F32, name="psk", tag="psk")
                    for r in range(4):
                        for kt in range(2):
                            xv = x_sb[:, kt, :].rearrange(
                                "p (blk cc jj) -> p blk cc jj", cc=4, jj=128)
                            nc.tensor.matmul(
                                ps[32 * r:32 * r + 32, :],
                                w_q[:, kt, u, 0:DIM_HEAD],
                                xv[:, 4 * h:4 * h + 4, r, :],
                                start=(kt == 0), stop=(kt == 1),
                                tile_position=(0, 32 * r))
                    sq = sc.tile([128, CHUNK], F16, name="sq", tag="sq")
                    qg_sb = sc.tile([128, CHUNK], F32, name="qg_sb",
                                    tag="qg_sb")
                    nc.vector.tensor_copy(qg_sb[:, :], ps[:, :])
                    nc.vector.tensor_mul(sq[:, :], qg_sb[:, :], qg_sb[:, :])
                    ps2 = pssq.tile([4, CHUNK], F32, name="psssq", tag="psssq")
                    nc.tensor.matmul(ps2[:, :], o4[:, :], sq[:, :],
                                     start=True, stop=True)
                    nc.vector.tensor_copy(
                        sstq[u][:, 4 * h:4 * h + 4, :],
                        ps2[:, :].rearrange("r (cc jj) -> r cc jj", jj=128))

                # k grouped projection + ssq
                for h in range(2):
                    ps = pk.tile([128, CHUNK], F32, name="psk", tag="psk")
                    for r in range(4):
                        for kt in range(2):
                            xv = x_sb[:, kt, :].rearrange(
                                "p (blk cc jj) -> p blk cc jj", cc=4, jj=128)
                            nc.tensor.matmul(
                                ps[32 * r:32 * r + 32, :],
                                w_k[:, kt, u, :],
                                xv[:, 4 * h:4 * h + 4, r, :],
                                start=(kt == 0), stop=(kt == 1),
                                tile_position=(0, 32 * r))
                    nc.vector.tensor_copy(
                        kg_raw[u][:, h * CHUNK:(h + 1) * CHUNK], ps[:, :])
                    sq = sc.tile([128, CHUNK], F16, name="sq", tag="sq")
                    kr = kg_raw[u][:, h * CHUNK:(h + 1) * CHUNK]
                    nc.vector.tensor_mul(sq[:, :], kr, kr)
                    ps2 = pssq.tile([4, CHUNK], F32, name="psssq", tag="psssq")
                    nc.tensor.matmul(ps2[:, :], o4[:, :], sq[:, :],
                                     start=True, stop=True)
                    nc.vector.tensor_copy(
                        sstk[u][:, 4 * h:4 * h + 4, :],
                        ps2[:, :].rearrange("r (cc jj) -> r cc jj", jj=128))

                nc.scalar.activation(sstq[u][:, :, :], sstq[u][:, :, :],
                                     mybir.ActivationFunctionType.Ln)
                nc.scalar.activation(sstk[u][:, :, :], sstk[u][:, :, :],
                                     mybir.ActivationFunctionType.Ln)
                # rq = 10*A_PRE/|q| = exp(-0.5*ln(ssq) + LNPRE);  rk = 1/|k|
                nc.scalar.activation(sstq[u][:, :, :], sstq[u][:, :, :],
                                     mybir.ActivationFunctionType.Exp,
                                     bias=lnpre_t[:, :], scale=-0.5)
                nc.scalar.activation(sstk[u][:, :, :], sstk[u][:, :, :],
                                     mybir.ActivationFunctionType.Exp,
                                     bias=0.0, scale=-0.5)
                for a, sst in ((0, sstq[u]), (1, sstk[u])):
                    nc.gpsimd.dma_start(
                        out=rr_d[u, a, :].rearrange("(c r jj) -> r c jj",
                                                    r=4, jj=128),
                        in_=sst[:, :, :])

                rqb = chains.tile([128, N], F32, name="rqb", tag="rqb")
                if u == 0:
                    # broadcast on GPSIMD, quartered for pipelining
                    rq_row = chains.tile([1, N], F32, name="rq_row",
                                         tag="rq_row")
                    nc.sync.dma_start(out=rq_row[:, :],
                                      in_=rr_d[u, 0, :].unsqueeze(0))
                    for hh in range(4):
                        hs = slice(hh * (N // 4), (hh + 1) * (N // 4))
                        nc.gpsimd.partition_broadcast(rqb[:, hs],
                                                      rq_row[:, hs])
                        nc.vector.tensor_mul(qs[u][:, hs], q_rep[u][:, hs],
                                             rqb[:, hs])
                else:
                    # broadcast-DMAs (overlap the running mainloop)
                    for ch in range(NCHUNK):
                        eng = nc.sync if ch % 2 == 0 else nc.gpsimd
                        eng.dma_start(
                            out=rqb[:, ch * CHUNK:(ch + 1) * CHUNK],
                            in_=rr_d[u, 0, ch * CHUNK:(ch + 1) * CHUNK]
                            .partition_broadcast(128))
                    for hh in range(4):
                        hs = slice(hh * (N // 4), (hh + 1) * (N // 4))
                        nc.vector.tensor_mul(qs[u][:, hs], q_rep[u][:, hs],
                                             rqb[:, hs])
                rkb = chains.tile([128, N // 4], F32, name="rkb", tag="rkb")
                rkv = rr_d[u, 1, :].rearrange("(c r jj) -> r c jj", r=4,
                                              jj=128)
                for r in range(4):
                    eng = nc.sync if r % 2 == 0 else nc.gpsimd
                    eng.dma_start(
                        out=rkb[32 * r:32 * r + 32, :].rearrange(
                            "p (c jj) -> p c jj", jj=128),
                        in_=rkv[r, :, :].partition_broadcast(32))
                for hh in range(2):
                    hs = slice(hh * (N // 8), (hh + 1) * (N // 8))
                    nc.vector.tensor_mul(kg[u][:, hs], kg_raw[u][:, hs],
                                         rkb[:, hs])

        # =========================== P2: attention =========================
        # PSUM banks: stA 0-2 (3), stB 3-4 (2), av 5, rs 6, y 7.
        with ExitStack() as p2:
            pstA = p2.enter_context(tc.tile_pool(name="pstA", bufs=1,
                                                 space="PSUM"))
            pstB = p2.enter_context(tc.tile_pool(name="pstB", bufs=1,
                                                 space="PSUM"))
            pav = p2.enter_context(tc.tile_pool(name="pav", bufs=1,
                                                space="PSUM"))
            prs = p2.enter_context(tc.tile_pool(name="prs", bufs=1,
                                                space="PSUM"))
            ptp = p2.enter_context(tc.tile_pool(name="ptp", bufs=2))
            sc2 = p2.enter_context(tc.tile_pool(name="p2scratch", bufs=2))

            rconst = RECIP_APPROX_FAST_CONSTS

            start_slot = 0
            for u in range(UNITS):
                for ch in range(NCHUNK):
                    i0 = ch * CHUNK
                    groups = _p2_groups(start_slot)
                    start_slot ^= (len(groups) % 2)
                    engines = _engine_plan(groups)
                    av = pav.tile([128, CHUNK], F32, name="av", tag="av")
                    rs = prs.tile([128, CHUNK], F32, name="rs", tag="rs")
                    if u == 0 and ch == 0:
                        # stale-PSUM guard: rows the rs matmuls never write
                        # must be finite for the sel-fold (0 * NaN = NaN)
                        nc.vector.memset(rs[:, :], 0.0)

                    emitted = []  # (st, pt, eng, jt0, g)
                    pending_avrs = []

                    def emit_exp(rec):
                        st, pt, eng, jt0, g = rec
                        stf = st[:, 0:g, :].rearrange("p g f -> p (g f)")
                        ptf = pt[:, 0:g, :].rearrange("p g f -> p (g f)")
                        if eng == "ACT":
                            nc.scalar.activation(
                                ptf, stf,
                                mybir.ActivationFunctionType.Exp,
                                bias=0.0, scale=LN2_1024)
                        else:
                            nc.vector._custom_dve(
                                FAST_EXP_OP, out=ptf, in0=stf,
                                in1=b2t[:, :], s0=EXP_B1, s1=EXP_C1,
                                imm2=EXP_MASK)

                    def emit_avrs(rec):
                        st, pt, eng, jt0, g = rec

                        def rhs(s):
                            if eng == "ACT":
                                return pt[:, s, :]
                            return pt[:, s, :].bitcast(F16).rearrange(
                                "p (n two) -> p n two", two=2)[:, :, 0]

                        for s in range(g):
                            j = jt0 + s
                            c4 = 32 * (j % 4)
                            nc.tensor.matmul(
                                av[c4:c4 + 32, :],
                                vt[u][:, j, :],
                                rhs(s),
                                start=(j < 4), stop=(j >= JT - 4),
                                tile_position=(0, c4))
                        for s in range(g):
                            j = jt0 + s
                            c4 = 32 * (j % 4)
                            nc.tensor.matmul(
                                rs[c4:c4 + 1, :],
                                ones_rs[:, :],
                                rhs(s),
                                start=(j < 4), stop=(j >= JT - 4),
                                tile_position=(0, c4))

                    jt0 = 0
                    for gidx, (g, slot) in enumerate(groups):
                        pool = pstA if slot == 0 else pstB
                        gmax = 3 if slot == 0 else 2
                        st = pool.tile([128, gmax, CHUNK], F32,
                                       name=f"st{slot}", tag=f"st{slot}")
                        if engines[gidx] == "ACT":
                            pt = ptp.tile([128, gmax, CHUNK], F16,
                                          name=f"ptA{slot}", tag=f"ptA{slot}")
                        else:
                            pt = ptp.tile([128, gmax, CHUNK], mybir.dt.int32,
                                          name=f"ptD{slot}", tag=f"ptD{slot}")
                        for s in range(g):
                            j = jt0 + s
                            r = j % 4
                            t = j // 4
                            nc.tensor.matmul(
                                st[:, s, :],
                                kg[u][32 * r:32 * r + 32,
                                      t * 128:(t + 1) * 128],
                                qs[u][32 * r:32 * r + 32, i0:i0 + CHUNK],
                                start=True, stop=True,
                                tile_position=(32 * r, 0))
                        emitted.append((st, pt, engines[gidx], jt0, g))
                        if len(emitted) >= 2:
                            emit_exp(emitted[-2])
                            pending_avrs.append(emitted[-2])
                        if len(pending_avrs) >= 2:
                            emit_avrs(pending_avrs.pop(0))
                        jt0 += g
                    emit_exp(emitted[-1])
                    pending_avrs.append(emitted[-1])
                    for rec in pending_avrs:
                        emit_avrs(rec)

                    # ---- chunk epilogue: rowsum fold + recip + oT scale ----
                    rs_sb = sc2.tile([128, CHUNK], F16, name="rs_sb",
                                     tag="rs_sb")
                    nc.scalar.copy(rs_sb[:, :], rs[:, :])
                    fold = prs.tile([128, CHUNK], F32, name="fold", tag="rs")
                    nc.tensor.matmul(fold[:, :], sel[:, :], rs_sb[:, :],
                                     start=True, stop=True)
                    rsb = sc2.tile([128, CHUNK], F32, name="rsb", tag="rsb")
                    nc.vector._custom_dve(
                        RECIPROCAL_APPROX_FAST, out=rsb[:, :], in0=fold[:, :],
                        s0=rconst["s0"], s1=rconst["s1"], imm2=rconst["imm2"])
                    nc.vector.tensor_mul(oT[u][:, i0:i0 + CHUNK], av[:, :],
                                         rsb[:, :])

        p12.close()

        # =========================== P3: output projection =================
        with ExitStack() as p3:
            sc3 = p3.enter_context(tc.tile_pool(name="p3scratch", bufs=2))
            py3 = p3.enter_context(tc.tile_pool(name="py3", bufs=4,
                                                space="PSUM"))
            engs = [nc.sync, nc.gpsimd, nc.scalar]
            for ch in range(NCHUNK):
                cs = slice(ch * CHUNK, (ch + 1) * CHUNK)
                for m in range(2):
                    ps = py3.tile([128, CHUNK], F32, name="psy", tag="psy")
                    for u in range(UNITS):
                        nc.tensor.matmul(
                            ps[:, :],
                            w_o[:, u, m, :],
                            oT[u][:, cs],
                            start=(u == 0), stop=(u == 1))
                    ysb = sc3.tile([128, CHUNK], F32, name="ysb", tag="ysb",
                                   bufs=4)
                    ceng = nc.vector if (ch + m) % 2 == 0 else nc.scalar
                    ceng.tensor_copy(ysb[:, :], ps[:, :]) \
                        if ceng is nc.vector else ceng.copy(ysb[:, :], ps[:, :])
                    engs[(2 * ch + m) % 3].dma_start(
                        out=y_out[m * 128:(m + 1) * 128, cs], in_=ysb[:, :])

    nc.compile()
    return nc


_NC_CACHE = None


def _get_nc():
    global _NC_CACHE
    if _NC_CACHE is None:
        _NC_CACHE = _build()
    return _NC_CACHE


def _make_in_maps(x, w_qkv, w_out):
    """Build the 8 per-core input dicts from full inputs."""
    x = np.ascontiguousarray(x, dtype=np.float32)
    w_qkv = np.ascontiguousarray(w_qkv, dtype=np.float32)
    w_out = np.ascontiguousarray(w_out, dtype=np.float32)
    b, c, h, w = x.shape
    xf = x.reshape(b, c, h * w)

    ones4 = np.zeros((128, 4), np.float16)
    for r in range(4):
        ones4[32 * r:32 * r + 32, r] = 1.0

    in_maps = []
    for core in range(NCORES):
        bb = core // 2
        p = core % 2
        heads = [2 * p, 2 * p + 1]
        wq = np.stack([w_qkv[hh * DIM_HEAD:(hh + 1) * DIM_HEAD, :]
                       for hh in heads])
        wk = np.stack([w_qkv[HIDDEN + hh * DIM_HEAD:
                             HIDDEN + (hh + 1) * DIM_HEAD, :] for hh in heads])
        wv = np.stack([w_qkv[2 * HIDDEN + hh * DIM_HEAD:
                             2 * HIDDEN + (hh + 1) * DIM_HEAD, :]
                       for hh in heads])
        wqT = np.ascontiguousarray(
            np.concatenate([np.transpose(wq, (0, 2, 1))] * 4, axis=2))
        wkT = np.ascontiguousarray(np.transpose(wk, (0, 2, 1)))
        wvT = np.ascontiguousarray(
            np.concatenate([wv[0].T, wv[1].T], axis=1)) * VS  # [256, 64]
        # wo4[u][32c+d, m, mm] = w_out[m*128+mm, head_u*32+d], replicated 4x
        wo4 = np.zeros((UNITS, 128, 2, 128), np.float32)
        for u, hh in enumerate(heads):
            blk = w_out[:, hh * DIM_HEAD:(hh + 1) * DIM_HEAD].T  # [32, 256]
            rep = np.tile(blk, (4, 1))                           # [128, 256]
            wo4[u] = rep.reshape(128, 2, 128)
        in_maps.append({
            "x_in": np.ascontiguousarray(xf[bb]).astype(np.float16),
            "wqT": wqT.astype(np.float16),
            "wkT": wkT.astype(np.float16),
            "wvT": wvT.astype(np.float16),
            "wo4": wo4.astype(np.float16),
            "ones4": ones4,
        })
    return in_maps


def kernel(x, w_qkv, w_out, b_out):
    nc = _get_nc()
    in_maps = _make_in_maps(x, w_qkv, w_out)
    res = run_bass_kernel_spmd(nc, in_maps, core_ids=list(range(NCORES)))
    outs = res.results
    y = np.zeros((B, C, N), np.float32)
    for bb in range(B):
        y[bb] = outs[2 * bb]["y_out"] + outs[2 * bb + 1]["y_out"]
    y += np.asarray(b_out, np.float32)[None, :, None]
    return y.reshape(B, C, H, W).astype(np.float32)
